# revision 10
# baseline (speedup 1.0000x reference)
"""MoE-LoRA double GEMM on 8 Trainium2 NeuronCores (bf16 pipeline).

Computes, for E=4 experts:  h_e = x @ A_e^T ; y_e = h_e @ B_e^T
with x:[4,2048,4096] f32, A:[4,64,4096], B:[4,4096,64] ->
y:[4,4,2048,4096].

Strategy: data-parallel shard x over tokens (8192 tokens -> 1024/core),
replicate the small expert weights.  All device I/O and matmul operands
are bf16 (PSUM accumulation stays fp32), which halves HBM traffic vs
fp32 (~46 MB/core) and runs the PE at 1 cycle/row instead of fp32r's
~4: the fp32 version of this kernel was simultaneously PE-bound (97%
matmul busy) and HBM-bound.

Pipeline notes (hard-won from traces):
- Tokens split [256, 256, 512] so first stores issue early; per pair p
  the flow is GEMM1(p) -> GEMM2(p).
- The first x tile loads on the *Sync* HWDGE ring, whose preamble ends
  ~2us in (Scalar's ACT-table/const preamble delays its first trigger
  to ~7us); remaining loads ride Scalar in A0, A1, B0, x_t1, B1, x_t2
  order.  Tile schedules engine FIFOs against a DMA cost model, so a
  weight arriving later than the model expects stalls the whole Tensor
  queue - every weight must land before its scheduled consumer.
- One shared 4-buf [128,1024] fp32 PSUM pool serves GEMM1 (half-tile)
  and GEMM2: in the long GEMM2-only tail the pool is effectively
  4-deep, which keeps matmuls from stalling on PSUM-evacuation and
  keeps the PE HAM-warm.
- PSUM->SBUF evacuation casts to bf16, alternating VectorE/ScalarE.
- Both experts of a pair stage into one [128, 2, O] tile and store as
  a single 2 MB DMA (dst rearranged "e t o -> t e o").
"""

import os
import sys

import numpy as np

for _p in ("/opt/trn_rl_repo", "/root/.axon_site/_ro/trn_rl_repo"):
    if os.path.isdir(_p) and _p not in sys.path:
        sys.path.append(_p)

import ml_dtypes

from concourse import bacc, mybir, tile
from concourse.bass_utils import run_bass_kernel_spmd

E = 4
R_E = 64
D = 4096
O = 4096
B_DIM = 4
S = 2048
T = B_DIM * S          # 8192 tokens total
NCORES = 8
TL = T // NCORES       # 1024 tokens per core
TTS = [256, 256, 512]  # pipeline tile sizes; sum == TL
NCD = D // 128         # 32 contraction chunks

FP32 = mybir.dt.float32
BF16 = mybir.dt.bfloat16
NP_BF16 = ml_dtypes.bfloat16

_CACHE = {}


def _build_nc():
    nc = bacc.Bacc(None, target_bir_lowering=False, debug=False)
    xt_d = nc.declare_dram_parameter("xT", [128, NCD * TL], BF16, isOutput=False)
    at_d = nc.declare_dram_parameter("AT", [2, 128, NCD, 128], BF16, isOutput=False)
    bt_d = nc.declare_dram_parameter("BT", [2, 128, O], BF16, isOutput=False)
    y_d = nc.declare_dram_parameter("y", [E, TL, O], BF16, isOutput=True)

    with tile.TileContext(nc) as tc:
        with (
            tc.tile_pool(name="atc", bufs=2) as atpool,
            tc.tile_pool(name="btc", bufs=2) as btpool,
            tc.tile_pool(name="xt", bufs=len(TTS)) as xtpool,
            tc.tile_pool(name="ht", bufs=6) as htpool,
            tc.tile_pool(name="ys", bufs=4) as yspool,
            tc.tile_pool(name="ps", bufs=4, space="PSUM") as pspool,
        ):
            atq = [None, None]
            btc = [None, None]
            xqs = [None] * len(TTS)

            def load_a(p):
                ac = atpool.tile([128, NCD, 128], BF16, name=f"at{p}", tag="atc")
                nc.scalar.dma_start(out=ac[:], in_=at_d[p])
                atq[p] = ac

            def load_b(p):
                bc = btpool.tile([128, O], BF16, name=f"bt{p}", tag="btc")
                nc.scalar.dma_start(out=bc[:], in_=bt_d[p])
                btc[p] = bc

            def load_x(tt, t0, TTi, eng):
                xc = xtpool.tile([128, NCD * TTi], BF16, name=f"xc{tt}", tag="xtc")
                eng.dma_start(out=xc[:], in_=xt_d[:, NCD * t0 : NCD * (t0 + TTi)])
                xqs[tt] = xc

            t_starts = [sum(TTS[:i]) for i in range(len(TTS))]
            load_x(0, t_starts[0], TTS[0], nc.sync)   # sync ring frees first
            load_a(0)
            load_b(0)
            load_a(1)
            load_x(1, t_starts[1], TTS[1], nc.scalar)
            load_b(1)
            load_x(2, t_starts[2], TTS[2], nc.scalar)

            copy_turn = [0]

            def copy_psum(dst, src):
                # alternate PSUM evacuation between VectorE and ScalarE
                if copy_turn[0] == 0:
                    nc.vector.tensor_copy(dst, src)
                else:
                    nc.scalar.copy(dst, src)
                copy_turn[0] ^= 1

            # alternate the store stream between the two HWDGE rings (Sync +
            # Scalar): a single HWDGE FIFO caps at ~320 GB/s from the
            # inter-transfer bubble; two queues round-robin at packet
            # granularity and together reach the ~358 GB/s HBM-per-NC limit.
            # (GpSimd SWDGE was tried here: its slow drain delayed ys-tile
            # recycling and stalled the supply pipeline.)
            store_turn = [0]

            def store_y(out, in_):
                if store_turn[0] == 0:
                    nc.sync.dma_start(out=out, in_=in_)
                else:
                    nc.scalar.dma_start(out=out, in_=in_)
                store_turn[0] ^= 1

            for tt, TTi in enumerate(TTS):
                t0 = t_starts[tt]
                xc = xqs[tt]
                TG = TTi // 128
                for p in range(2):
                    # ---- GEMM1: h[pair] = A-packed^T @ x^T, fp32 accum ----
                    phtt = pspool.tile([128, 1024], FP32, name=f"pht{tt}_{p}", tag="ps")
                    pht = phtt[:, :512]
                    for c in range(NCD):
                        nc.tensor.matmul(
                            pht[:, :TTi],
                            atq[p][:, c, :],
                            xc[:, c * TTi : (c + 1) * TTi],
                            start=(c == 0),
                            stop=(c == NCD - 1),
                        )
                    ht = htpool.tile([128, TTi], BF16, name=f"ht{tt}_{p}", tag="ht")
                    copy_psum(ht[:], pht[:, :TTi])

                    # ---- GEMM2: y_e = h_e^T @ B_e^T, strip-parallel ----
                    for g in range(TG):
                        ys = yspool.tile(
                            [128, 2, O], BF16, name=f"ys{tt}_{p}_{g}", tag="ys"
                        )
                        for q in range(O // 1024):
                            pys = [
                                pspool.tile([128, 1024], FP32, name=f"py{s}", tag="ps")
                                for s in range(2)
                            ]
                            for j in range(2):
                                oc = 2 * q + j
                                for s in range(2):
                                    r0 = 64 * s
                                    nc.tensor.matmul(
                                        pys[s][:, j * 512 : (j + 1) * 512],
                                        ht[r0 : r0 + 64, g * 128 : (g + 1) * 128],
                                        btc[p][
                                            r0 : r0 + 64, oc * 512 : (oc + 1) * 512
                                        ],
                                        start=True,
                                        stop=True,
                                    )
                            for s in range(2):
                                copy_psum(
                                    ys[:, s, q * 1024 : (q + 1) * 1024], pys[s][:]
                                )
                        store_y(
                            y_d[
                                2 * p : 2 * p + 2, t0 + g * 128 : t0 + (g + 1) * 128, :
                            ].rearrange("e t o -> t e o"),
                            ys[:],
                        )
    nc.compile()
    return nc


def _get_nc():
    if "nc" not in _CACHE:
        _CACHE["nc"] = _build_nc()
    return _CACHE["nc"]


def _prep_weights(A, B):
    A = np.asarray(A, dtype=np.float32)
    B = np.asarray(B, dtype=np.float32)
    at = np.empty((2, 128, NCD, 128), dtype=NP_BF16)
    bt = np.empty((2, 128, O), dtype=NP_BF16)
    for p in range(2):
        # stationary for GEMM1: [D, 128] with expert 2p in cols 0-63, 2p+1 in 64-127
        atp = np.concatenate([A[2 * p].T, A[2 * p + 1].T], axis=1)  # [4096, 128]
        at[p] = atp.reshape(NCD, 128, 128).transpose(1, 0, 2).astype(NP_BF16)
        # moving for GEMM2: [128, O] with expert 2p rows 0-63, 2p+1 rows 64-127
        bt[p] = np.concatenate([B[2 * p].T, B[2 * p + 1].T], axis=0).astype(NP_BF16)
    return at, bt


def kernel(x, A, B, _trace=False):
    x = np.asarray(x, dtype=np.float32)
    # per-core, tile-major bf16 layout: [core][p][tile: c-major, t-minor]
    xb = x.reshape(NCORES, TL, NCD, 128).astype(NP_BF16)
    xh_all = np.empty((NCORES, 128, NCD * TL), dtype=NP_BF16)
    t_starts = [sum(TTS[:i]) for i in range(len(TTS))]
    for tt, TTi in enumerate(TTS):
        t0 = t_starts[tt]
        # [core, t, c, p] -> [core, p, c, t]
        blk = xb[:, t0 : t0 + TTi].transpose(0, 3, 2, 1)
        xh_all[:, :, NCD * t0 : NCD * (t0 + TTi)] = blk.reshape(
            NCORES, 128, NCD * TTi
        )
    at, bt = _prep_weights(A, B)

    nc = _get_nc()
    in_maps = [{"xT": xh_all[k], "AT": at, "BT": bt} for k in range(NCORES)]
    res = run_bass_kernel_spmd(nc, in_maps, list(range(NCORES)), trace=_trace)
    if _trace:
        _CACHE["last_result"] = res

    y = np.empty((E, T, O), dtype=np.float32)
    for k in range(NCORES):
        y[:, k * TL : (k + 1) * TL, :] = res.results[k]["y"].astype(np.float32)
    return y.reshape(E, B_DIM, S, O)


# revision 11
# speedup vs baseline: 1.1375x; 1.1375x over previous
"""MoE-LoRA double GEMM on 8 Trainium2 NeuronCores (bf16 pipeline).

Computes, for E=4 experts:  h_e = x @ A_e^T ; y_e = h_e @ B_e^T
with x:[4,2048,4096] f32, A:[4,64,4096], B:[4,4096,64] ->
y:[4,4,2048,4096].

Strategy: data-parallel shard x over tokens (8192 tokens -> 1024/core),
replicate the small expert weights.  All device I/O and matmul operands
are bf16 (PSUM accumulation stays fp32), which halves HBM traffic vs
fp32 (~46 MB/core) and runs the PE at 1 cycle/row instead of fp32r's
~4: the fp32 version of this kernel was simultaneously PE-bound (97%
matmul busy) and HBM-bound.

Pipeline notes (hard-won from traces):
- Tokens split [256, 256, 512] so first stores issue early; per pair p
  the flow is GEMM1(p) -> GEMM2(p).
- The first x tile loads on the *Sync* HWDGE ring, whose preamble ends
  ~2us in (Scalar's ACT-table/const preamble delays its first trigger
  to ~7us); remaining loads ride Scalar in A0, A1, B0, x_t1, B1, x_t2
  order.  Tile schedules engine FIFOs against a DMA cost model, so a
  weight arriving later than the model expects stalls the whole Tensor
  queue - every weight must land before its scheduled consumer.
- One shared 4-buf [128,1024] fp32 PSUM pool serves GEMM1 (half-tile)
  and GEMM2: in the long GEMM2-only tail the pool is effectively
  4-deep, which keeps matmuls from stalling on PSUM-evacuation and
  keeps the PE HAM-warm.
- PSUM->SBUF evacuation casts to bf16, alternating VectorE/ScalarE.
- Both experts of a pair stage into one [128, 2, O] tile and store as
  a single 2 MB DMA (dst rearranged "e t o -> t e o").
"""

import os
import sys

import numpy as np

for _p in ("/opt/trn_rl_repo", "/root/.axon_site/_ro/trn_rl_repo"):
    if os.path.isdir(_p) and _p not in sys.path:
        sys.path.append(_p)

import ml_dtypes

from concourse import bacc, mybir, tile
from concourse.bass_utils import run_bass_kernel_spmd

E = 4
R_E = 64
D = 4096
O = 4096
B_DIM = 4
S = 2048
T = B_DIM * S          # 8192 tokens total
NCORES = 8
TL = T // NCORES       # 1024 tokens per core
TTS = [256, 256, 512]  # pipeline tile sizes; sum == TL
NCD = D // 128         # 32 contraction chunks

FP32 = mybir.dt.float32
BF16 = mybir.dt.bfloat16
NP_BF16 = ml_dtypes.bfloat16

_CACHE = {}


def _build_nc():
    nc = bacc.Bacc(None, target_bir_lowering=False, debug=False)
    xt_d = nc.declare_dram_parameter("xT", [128, NCD * TL], BF16, isOutput=False)
    at_d = nc.declare_dram_parameter("AT", [2, 128, NCD, 128], BF16, isOutput=False)
    bt_d = nc.declare_dram_parameter("BT", [2, 128, O], BF16, isOutput=False)
    y_d = nc.declare_dram_parameter("y", [E, TL, O], BF16, isOutput=True)

    with tile.TileContext(nc) as tc:
        with (
            tc.tile_pool(name="atc", bufs=2) as atpool,
            tc.tile_pool(name="btc", bufs=2) as btpool,
            tc.tile_pool(name="xt", bufs=len(TTS)) as xtpool,
            tc.tile_pool(name="ht", bufs=6) as htpool,
            tc.tile_pool(name="ys", bufs=4) as yspool,
            tc.tile_pool(name="ps", bufs=4, space="PSUM") as pspool,
        ):
            atq = [None, None]
            btc = [None, None]
            xqs = [None] * len(TTS)

            def load_a(p):
                ac = atpool.tile([128, NCD, 128], BF16, name=f"at{p}", tag="atc")
                nc.scalar.dma_start(out=ac[:], in_=at_d[p])
                atq[p] = ac

            def load_b(p):
                bc = btpool.tile([128, O], BF16, name=f"bt{p}", tag="btc")
                nc.scalar.dma_start(out=bc[:], in_=bt_d[p])
                btc[p] = bc

            def load_x(tt, t0, TTi, eng):
                xc = xtpool.tile([128, NCD * TTi], BF16, name=f"xc{tt}", tag="xtc")
                eng.dma_start(out=xc[:], in_=xt_d[:, NCD * t0 : NCD * (t0 + TTi)])
                xqs[tt] = xc

            t_starts = [sum(TTS[:i]) for i in range(len(TTS))]
            load_x(0, t_starts[0], TTS[0], nc.sync)   # sync ring frees first
            load_a(0)
            load_b(0)
            load_a(1)
            load_x(1, t_starts[1], TTS[1], nc.scalar)
            load_b(1)
            load_x(2, t_starts[2], TTS[2], nc.scalar)

            copy_turn = [0]

            def copy_psum(dst, src):
                # alternate PSUM evacuation between VectorE and ScalarE
                if copy_turn[0] == 0:
                    nc.vector.tensor_copy(dst, src)
                else:
                    nc.scalar.copy(dst, src)
                copy_turn[0] ^= 1

            # split the store stream between the Sync HWDGE ring and the
            # GpSimd SWDGE ring 2:1: a single HWDGE FIFO caps at ~320 GB/s
            # from the inter-transfer bubble, so a second queue is needed to
            # reach the ~358 GB/s HBM-per-NC limit, but SWDGE drains slowly
            # so it only gets every third store (Scalar's HWDGE ring is no
            # alternative - store triggers there stall behind ring
            # backpressure and block the Scalar copy stream).
            store_turn = [0]

            def store_y(out, in_):
                if store_turn[0] == 2:
                    nc.gpsimd.dma_start(out=out, in_=in_)
                else:
                    nc.sync.dma_start(out=out, in_=in_)
                store_turn[0] = (store_turn[0] + 1) % 3

            for tt, TTi in enumerate(TTS):
                t0 = t_starts[tt]
                xc = xqs[tt]
                TG = TTi // 128
                for p in range(2):
                    # ---- GEMM1: h[pair] = A-packed^T @ x^T, fp32 accum ----
                    phtt = pspool.tile([128, 1024], FP32, name=f"pht{tt}_{p}", tag="ps")
                    pht = phtt[:, :512]
                    for c in range(NCD):
                        nc.tensor.matmul(
                            pht[:, :TTi],
                            atq[p][:, c, :],
                            xc[:, c * TTi : (c + 1) * TTi],
                            start=(c == 0),
                            stop=(c == NCD - 1),
                        )
                    ht = htpool.tile([128, TTi], BF16, name=f"ht{tt}_{p}", tag="ht")
                    copy_psum(ht[:], pht[:, :TTi])

                    # ---- GEMM2: y_e = h_e^T @ B_e^T, strip-parallel ----
                    for g in range(TG):
                        ys = yspool.tile(
                            [128, 2, O], BF16, name=f"ys{tt}_{p}_{g}", tag="ys"
                        )
                        for q in range(O // 1024):
                            pys = [
                                pspool.tile([128, 1024], FP32, name=f"py{s}", tag="ps")
                                for s in range(2)
                            ]
                            for j in range(2):
                                oc = 2 * q + j
                                for s in range(2):
                                    r0 = 64 * s
                                    nc.tensor.matmul(
                                        pys[s][:, j * 512 : (j + 1) * 512],
                                        ht[r0 : r0 + 64, g * 128 : (g + 1) * 128],
                                        btc[p][
                                            r0 : r0 + 64, oc * 512 : (oc + 1) * 512
                                        ],
                                        start=True,
                                        stop=True,
                                    )
                            for s in range(2):
                                copy_psum(
                                    ys[:, s, q * 1024 : (q + 1) * 1024], pys[s][:]
                                )
                        store_y(
                            y_d[
                                2 * p : 2 * p + 2, t0 + g * 128 : t0 + (g + 1) * 128, :
                            ].rearrange("e t o -> t e o"),
                            ys[:],
                        )
    nc.compile()
    return nc


def _get_nc():
    if "nc" not in _CACHE:
        _CACHE["nc"] = _build_nc()
    return _CACHE["nc"]


def _prep_weights(A, B):
    A = np.asarray(A, dtype=np.float32)
    B = np.asarray(B, dtype=np.float32)
    at = np.empty((2, 128, NCD, 128), dtype=NP_BF16)
    bt = np.empty((2, 128, O), dtype=NP_BF16)
    for p in range(2):
        # stationary for GEMM1: [D, 128] with expert 2p in cols 0-63, 2p+1 in 64-127
        atp = np.concatenate([A[2 * p].T, A[2 * p + 1].T], axis=1)  # [4096, 128]
        at[p] = atp.reshape(NCD, 128, 128).transpose(1, 0, 2).astype(NP_BF16)
        # moving for GEMM2: [128, O] with expert 2p rows 0-63, 2p+1 rows 64-127
        bt[p] = np.concatenate([B[2 * p].T, B[2 * p + 1].T], axis=0).astype(NP_BF16)
    return at, bt


def kernel(x, A, B, _trace=False):
    x = np.asarray(x, dtype=np.float32)
    # per-core, tile-major bf16 layout: [core][p][tile: c-major, t-minor]
    xb = x.reshape(NCORES, TL, NCD, 128).astype(NP_BF16)
    xh_all = np.empty((NCORES, 128, NCD * TL), dtype=NP_BF16)
    t_starts = [sum(TTS[:i]) for i in range(len(TTS))]
    for tt, TTi in enumerate(TTS):
        t0 = t_starts[tt]
        # [core, t, c, p] -> [core, p, c, t]
        blk = xb[:, t0 : t0 + TTi].transpose(0, 3, 2, 1)
        xh_all[:, :, NCD * t0 : NCD * (t0 + TTi)] = blk.reshape(
            NCORES, 128, NCD * TTi
        )
    at, bt = _prep_weights(A, B)

    nc = _get_nc()
    in_maps = [{"xT": xh_all[k], "AT": at, "BT": bt} for k in range(NCORES)]
    res = run_bass_kernel_spmd(nc, in_maps, list(range(NCORES)), trace=_trace)
    if _trace:
        _CACHE["last_result"] = res

    y = np.empty((E, T, O), dtype=np.float32)
    for k in range(NCORES):
        y[:, k * TL : (k + 1) * TL, :] = res.results[k]["y"].astype(np.float32)
    return y.reshape(E, B_DIM, S, O)
